# revision 4
# baseline (speedup 1.0000x reference)
"""Paged GQA decode attention (sparse_attention) on 8 TRN2 NeuronCores.

Slot-parallel streaming design: the page table is (in the graded problem) a
permutation of all 32768 cache slots, so the full K/V caches are read exactly
once.  Instead of gathering each sequence's scattered 4KB rows (software-DGE
descriptor generation on gpsimd was nearly as expensive as the DMA itself and
the two serialized), core i owns the contiguous slot range
[i*4096, (i+1)*4096) and STREAMS it sequentially at full HBM bandwidth, in
chunks of 512 rows laid out [128 partitions, 4 rows, 1024] so every DMA
descriptor moves 16KB of contiguous DRAM per partition.

Each core scores its slots against ALL 16 sequences' queries, multiplies by a
host-built per-(slot, seq) ownership weight (0/1 under the permutation; counts
for general inputs), and accumulates the partial attention numerator (PV) and
denominator (row sums of masked exp) in PSUM over the whole stream.  The 16
new-token k/v rows are appended as a final zero-padded 128-row tile whose mask
rows inject each new token exactly once across the fleet; the streamed mask
row of a replaced slot is zeroed on the core that owns it.  No gathers, no
scatters, no cross-tile dependencies -- one fully static program per core.

PSUM accumulators are memset once and all PV/den matmuls run with
start=stop=False (skip_group_check): TRN2 zeroes lazily per 2KB region on
start_tensor_calc, so 8 per-head accumulation groups in one bank would
clobber each other; explicit zeroing + pure accumulation avoids that.

The host combines the 8 partial (num, den) pairs -- the split-K softmax
combine of flash-decoding -- and normalizes: out = sum_c num_c / sum_c den_c.
"""

import ml_dtypes
import numpy as np

# ---- problem constants (must match the harness's reference.py) ----
NUM_HEADS = 32
NUM_KV_HEADS = 8
HEAD_DIM = 128
BS = 16
KV_LEN = 2048
NUM_SLOTS = BS * KV_LEN          # 32768
D = NUM_KV_HEADS * HEAD_DIM      # 1024 (cache row width, f32)
SCALE = HEAD_DIM ** -0.5
N_CORES = 8
GROUP = NUM_HEADS // NUM_KV_HEADS  # 4

SLOTS_PER_CORE = NUM_SLOTS // N_CORES   # 4096
R = 4                                   # DRAM rows per partition per chunk
CHUNK = 128 * R                         # 512 slots per streamed chunk
NCHUNKS = SLOTS_PER_CORE // CHUNK       # 8
NT = SLOTS_PER_CORE // 128              # 32 streamed 128-slot groups
T = NT + 1                              # + appended new-token group
QCOLS = NUM_KV_HEADS * BS * GROUP       # 512 score columns: (h, b, g)


def build_program(mask_4d=True):
    import concourse.bacc as bacc
    import concourse.mybir as mybir
    import concourse.tile as tile

    f32 = mybir.dt.float32
    f32r = mybir.dt.float32r
    bf16 = mybir.dt.bfloat16
    EXP = mybir.ActivationFunctionType.Exp
    MULT = mybir.AluOpType.mult

    nc = bacc.Bacc("TRN2", target_bir_lowering=False, debug=False,
                   enable_asserts=False, num_devices=N_CORES,
                   num_swdge_queues=1)

    kc = nc.dram_tensor("k_shard", [SLOTS_PER_CORE, D], f32r,
                        kind="ExternalInput").ap()
    vc = nc.dram_tensor("v_shard", [SLOTS_PER_CORE, D], f32r,
                        kind="ExternalInput").ap()
    knew_d = nc.dram_tensor("k_new", [128, D], f32r, kind="ExternalInput").ap()
    vnew_d = nc.dram_tensor("v_new", [128, D], f32r, kind="ExternalInput").ap()
    qT_d = nc.dram_tensor("qT", [HEAD_DIM, QCOLS], bf16,
                          kind="ExternalInput").ap()
    mask_d = nc.dram_tensor("mask", [128, T * BS], f32r,
                            kind="ExternalInput").ap()
    ident_d = nc.dram_tensor("ident", [128, 128], f32r,
                             kind="ExternalInput").ap()
    num_d = nc.dram_tensor("num", [HEAD_DIM, QCOLS], f32,
                           kind="ExternalOutput").ap()
    den_d = nc.dram_tensor("den", [1, QCOLS], f32, kind="ExternalOutput").ap()

    kc_v = kc.rearrange("(c p j) d -> c p (j d)", p=128, j=R)
    vc_v = vc.rearrange("(c p j) d -> c p (j d)", p=128, j=R)

    with tile.TileContext(nc) as tc:
        with tc.tile_pool(name="const", bufs=1) as constp, \
             tc.tile_pool(name="kbuf", bufs=2) as kpool, \
             tc.tile_pool(name="vbuf", bufs=2) as vpool, \
             tc.tile_pool(name="ktsb", bufs=2) as ktp, \
             tc.tile_pool(name="psb", bufs=4) as ppool, \
             tc.tile_pool(name="outs", bufs=1) as outp, \
             tc.tile_pool(name="ps_kt", bufs=2, space="PSUM") as ps_kt, \
             tc.tile_pool(name="ps_s", bufs=2, space="PSUM") as ps_s, \
             tc.tile_pool(name="ps_pv", bufs=1, space="PSUM") as ps_pv, \
             tc.tile_pool(name="ps_den", bufs=1, space="PSUM") as ps_den:

            qt_sb = constp.tile([128, QCOLS], bf16)
            nc.sync.dma_start(qt_sb[:], qT_d)
            mask_sb = constp.tile([128, T * BS], f32r)
            nc.sync.dma_start(mask_sb[:], mask_d)
            ident = constp.tile([128, 128], f32r)
            nc.sync.dma_start(ident[:], ident_d)
            ones = constp.tile([128, 1], f32r)
            nc.vector.memset(ones[:].bitcast(f32), 1.0)

            pv = ps_pv.tile([128, QCOLS], f32, name="pv")
            nc.vector.memset(pv[:], 0.0)
            den = ps_den.tile([1, QCOLS], f32, name="den")
            nc.vector.memset(den[:], 0.0)

            def group(kbuf, vbuf, j, t):
                """one 128-slot group: slots live at kbuf[:, j, :]."""
                ktsb = ktp.tile([128, NUM_KV_HEADS, 128], bf16, tag="kt")
                for hg in range(2):
                    ktps = ps_kt.tile([128, 512], f32r, tag="ktps")
                    for i in range(4):
                        h = hg * 4 + i
                        nc.tensor.transpose(
                            ktps[:, i * 128:(i + 1) * 128],
                            kbuf[:, j, h * 128:(h + 1) * 128],
                            ident[:])
                    dst = ktsb[:, hg * 4:hg * 4 + 4, :]
                    src = ktps[:].rearrange("p (i d) -> p i d", d=128)
                    if hg == 0:
                        nc.vector.tensor_copy(dst, src)
                    else:
                        nc.scalar.copy(dst, src)

                # scores[slot, (h,b,g)] = sum_d K[slot,d_h] Q[(b,g),d]
                scores = ps_s.tile([128, QCOLS], f32, tag="scores")
                for h in range(NUM_KV_HEADS):
                    nc.tensor.matmul(
                        out=scores[:, h * 64:(h + 1) * 64],
                        lhsT=ktsb[:, h, :],
                        rhs=qt_sb[:, h * 64:(h + 1) * 64],
                        start=True, stop=True)

                p_sb = ppool.tile([128, QCOLS], f32r, tag="p")
                nc.scalar.activation(p_sb[:], scores[:], EXP, scale=SCALE)

                # ownership weights: pm[slot, (h,b,g)] = p * mask[slot, b]
                pm = ppool.tile([128, QCOLS], f32r, tag="pm")
                mcols = mask_sb[:, t * BS:(t + 1) * BS]
                if mask_4d:
                    nc.vector.tensor_tensor(
                        out=pm[:].rearrange("p (h b g) -> p h b g",
                                            h=NUM_KV_HEADS, b=BS, g=GROUP),
                        in0=p_sb[:].rearrange("p (h b g) -> p h b g",
                                              h=NUM_KV_HEADS, b=BS, g=GROUP),
                        in1=mcols.unsqueeze(1).unsqueeze(3).broadcast_to(
                            [128, NUM_KV_HEADS, BS, GROUP]),
                        op=MULT)
                else:
                    for h in range(NUM_KV_HEADS):
                        nc.vector.tensor_tensor(
                            out=pm[:, h * 64:(h + 1) * 64].rearrange(
                                "p (b g) -> p b g", g=GROUP),
                            in0=p_sb[:, h * 64:(h + 1) * 64].rearrange(
                                "p (b g) -> p b g", g=GROUP),
                            in1=mcols.unsqueeze(2).broadcast_to(
                                [128, BS, GROUP]),
                            op=MULT)

                # num[d, (h,b,g)] += sum_slot V[slot, d_h] * pm[slot, (b,g)]
                for h in range(NUM_KV_HEADS):
                    nc.tensor.matmul(
                        out=pv[:, h * 64:(h + 1) * 64],
                        lhsT=vbuf[:, j, h * 128:(h + 1) * 128],
                        rhs=pm[:, h * 64:(h + 1) * 64],
                        start=False, stop=False, skip_group_check=True)
                # den[(h,b,g)] += sum_slot pm
                nc.tensor.matmul(
                    out=den[0:1, :],
                    lhsT=ones[:],
                    rhs=pm[:],
                    start=False, stop=False, skip_group_check=True)

            for c in range(NCHUNKS):
                kbuf = kpool.tile([128, R, D], f32r, tag="k")
                nc.sync.dma_start(kbuf[:].rearrange("p j d -> p (j d)"),
                                  kc_v[c])
                vbuf = vpool.tile([128, R, D], f32r, tag="v")
                nc.scalar.dma_start(vbuf[:].rearrange("p j d -> p (j d)"),
                                    vc_v[c])
                for j in range(R):
                    group(kbuf, vbuf, j, c * R + j)

            kbuf = kpool.tile([128, 1, D], f32r, tag="knew")
            nc.sync.dma_start(kbuf[:, 0, :], knew_d)
            vbuf = vpool.tile([128, 1, D], f32r, tag="vnew")
            nc.scalar.dma_start(vbuf[:, 0, :], vnew_d)
            group(kbuf, vbuf, 0, NT)

            onum = outp.tile([128, QCOLS], f32)
            nc.vector.tensor_copy(onum[:], pv[:])
            nc.sync.dma_start(num_d, onum[:])
            oden = outp.tile([1, QCOLS], f32)
            nc.scalar.copy(oden[:], den[0:1, :])
            nc.scalar.dma_start(den_d, oden[:])

    nc.compile()
    return nc


def shard_inputs(q, k, v, k_cache, v_cache, slot_mapping, page_indices):
    """Host-side sharding: contiguous zero-copy cache slices per core plus
    small index-derived tensors (masks, transposed queries, new-token tiles)."""
    q = np.ascontiguousarray(np.asarray(q, dtype=np.float32))
    k = np.ascontiguousarray(np.asarray(k, dtype=np.float32))
    v = np.ascontiguousarray(np.asarray(v, dtype=np.float32))
    k_cache = np.asarray(k_cache, dtype=np.float32)
    v_cache = np.asarray(v_cache, dtype=np.float32)
    slot_mapping = np.asarray(slot_mapping, dtype=np.int64).ravel()
    page_indices = np.asarray(page_indices, dtype=np.int64)

    # qT[d, (h, b, g)] = q[b, h*GROUP+g, d]
    qr = q.reshape(BS, NUM_KV_HEADS, GROUP, HEAD_DIM)
    qT = np.ascontiguousarray(
        qr.transpose(3, 1, 0, 2).reshape(HEAD_DIM, QCOLS)
    ).astype(ml_dtypes.bfloat16)

    # ownership weights: count[slot, b] = multiplicity of slot in seq b's pages
    count = np.zeros((NUM_SLOTS, BS), dtype=np.float32)
    np.add.at(count,
              (page_indices.ravel(),
               np.repeat(np.arange(BS), KV_LEN)),
              1.0)
    # new-token slots: reference scatters k/v rows there BEFORE the gather, so
    # the streamed (old) row must contribute nothing; the appended kvnew tile
    # re-injects each referencing (seq, count) exactly once fleet-wide.
    # With duplicate slot_mapping entries the last writer wins (jax .at[].set).
    final_writer = {}
    for j in range(BS):
        final_writer[int(slot_mapping[j])] = j
    newcnt = np.zeros((BS, BS), dtype=np.float32)   # [kvnew row j, seq b]
    for s, j in final_writer.items():
        newcnt[j, :] = count[s, :]
        count[s, :] = 0.0

    # appended tile data: rows 0..15 = new k/v, rest zeros
    knew = np.zeros((128, D), dtype=np.float32)
    knew[:BS] = k
    vnew = np.zeros((128, D), dtype=np.float32)
    vnew[:BS] = v

    in_maps = []
    for c in range(N_CORES):
        base = c * SLOTS_PER_CORE
        # streamed slot (chunk ch, partition p, sub j) = ch*CHUNK + p*R + j
        # SBUF mask layout [p, t*16 + b], t = ch*R + j
        mcore = count[base:base + SLOTS_PER_CORE]          # [4096, 16]
        m = np.zeros((128, T * BS), dtype=np.float32)
        m[:, :NT * BS] = (
            mcore.reshape(NCHUNKS, 128, R, BS).transpose(1, 0, 2, 3)
            .reshape(128, NT * BS))
        # new-token tile mask: kvnew row j handled by core j % N_CORES
        for j in range(BS):
            if j % N_CORES == c:
                m[j, NT * BS:(NT + 1) * BS] = newcnt[j]
        in_maps.append({
            "k_shard": k_cache[base:base + SLOTS_PER_CORE],
            "v_shard": v_cache[base:base + SLOTS_PER_CORE],
            "k_new": knew,
            "v_new": vnew,
            "qT": qT,
            "mask": np.ascontiguousarray(m),
            "ident": np.eye(128, dtype=np.float32),
        })
    return in_maps


_PROGS = {}
last_results = None  # BassKernelResults of the most recent kernel() call


def kernel(q, k, v, k_cache, v_cache, slot_mapping, page_indices):
    global last_results
    from concourse.bass_utils import run_bass_kernel_spmd

    in_maps = shard_inputs(q, k, v, k_cache, v_cache, slot_mapping,
                           page_indices)
    if "prog" not in _PROGS:
        try:
            _PROGS["prog"] = build_program(mask_4d=True)
        except Exception:
            _PROGS["prog"] = build_program(mask_4d=False)
    res = run_bass_kernel_spmd(_PROGS["prog"], in_maps,
                               core_ids=list(range(N_CORES)))
    last_results = res

    num = np.zeros((HEAD_DIM, NUM_KV_HEADS, BS, GROUP), dtype=np.float64)
    den = np.zeros((NUM_KV_HEADS, BS, GROUP), dtype=np.float64)
    for c in range(N_CORES):
        num += res.results[c]["num"].astype(np.float64).reshape(
            HEAD_DIM, NUM_KV_HEADS, BS, GROUP)
        den += res.results[c]["den"].astype(np.float64).reshape(
            NUM_KV_HEADS, BS, GROUP)
    o = num / den                                   # [d, h, b, g]
    out = o.transpose(2, 1, 3, 0).reshape(BS, NUM_HEADS * HEAD_DIM)
    return np.ascontiguousarray(out.astype(np.float32))


# revision 5
# speedup vs baseline: 1.2567x; 1.2567x over previous
"""Paged GQA decode attention (sparse_attention) on 8 TRN2 NeuronCores.

Slot-parallel streaming design: the page table is (in the graded problem) a
permutation of all 32768 cache slots, so the full K/V caches are read exactly
once.  Core i owns the contiguous slot range [i*4096, (i+1)*4096) and STREAMS
it sequentially at full HBM bandwidth in chunks laid out
[128 partitions, r rows, 1024] (16KB contiguous DRAM per partition per
descriptor at r=4; the first chunks are smaller to spin the pipeline up fast).

Each core scores its slots against ALL 16 sequences' queries, multiplies by a
host-built per-(slot, seq) ownership weight (0/1 under the permutation;
counts for general inputs), and accumulates the partial attention numerator
and denominator in PSUM over the whole stream.  The 16 new-token k/v rows are
appended as a final zero-padded 128-row tile whose mask rows inject each new
token exactly once across the fleet; the streamed mask row of a replaced slot
is zeroed on the core that owns it.  No gathers, no scatters -- one fully
static program per core.  The host sums the per-core partials (flash-decoding
split-K combine) and normalizes.

Per-instruction HW profiling showed the tensor engine is cadence-bound at
roughly max(ldweights, matmul-stream) ~ 1ns/column, so the pipeline is built
around minimizing PE columns moved:
  - everything on the PE is bf16: K and V tiles are converted f32->bf16 on
    the Activation/Vector engines one group ahead (f32r narrow matmuls run
    at 4 cycles/row and f32r weight loads at ~2 cycles/column),
  - PV is flipped (lhsT = masked-P, 64-column weight loads; V is the moving
    operand) and V carries an appended ones-column per head so each PV
    matmul also produces that head's denominator -- no separate den matmul,
  - PSUM accumulators are memset once and PV runs start=stop=False
    (skip_group_check): TRN2 zeroes lazily per 2KB region on
    start_tensor_calc, so 8 per-head groups in one bank would clobber each
    other.
"""

import ml_dtypes
import numpy as np

# ---- problem constants (must match the harness's reference.py) ----
NUM_HEADS = 32
NUM_KV_HEADS = 8
HEAD_DIM = 128
BS = 16
KV_LEN = 2048
NUM_SLOTS = BS * KV_LEN          # 32768
D = NUM_KV_HEADS * HEAD_DIM      # 1024 (cache row width, f32)
SCALE = HEAD_DIM ** -0.5
N_CORES = 8
GROUP = NUM_HEADS // NUM_KV_HEADS  # 4

SLOTS_PER_CORE = NUM_SLOTS // N_CORES   # 4096
RMAX = 4                                # max DRAM rows per partition per chunk
# per-chunk rows/partition: small first chunks fill the pipeline quickly
RS = [1, 1, 2, 4, 4, 4, 4, 4, 4, 4]
assert sum(RS) == SLOTS_PER_CORE // 128
NT = SLOTS_PER_CORE // 128              # 32 streamed 128-slot groups
T = NT + 1                              # + appended new-token group
QCOLS = NUM_KV_HEADS * BS * GROUP       # 512 score columns: (h, b, g)
H = NUM_KV_HEADS


def build_program(mask_4d=True):
    import concourse.bacc as bacc
    import concourse.mybir as mybir
    import concourse.tile as tile

    f32 = mybir.dt.float32
    f32r = mybir.dt.float32r
    bf16 = mybir.dt.bfloat16
    EXP = mybir.ActivationFunctionType.Exp
    MULT = mybir.AluOpType.mult

    nc = bacc.Bacc("TRN2", target_bir_lowering=False, debug=False,
                   enable_asserts=False, num_devices=N_CORES,
                   num_swdge_queues=1)

    kc = nc.dram_tensor("k_shard", [SLOTS_PER_CORE, D], f32r,
                        kind="ExternalInput").ap()
    vc = nc.dram_tensor("v_shard", [SLOTS_PER_CORE, D], f32r,
                        kind="ExternalInput").ap()
    knew_d = nc.dram_tensor("k_new", [128, D], f32r, kind="ExternalInput").ap()
    vnew_d = nc.dram_tensor("v_new", [128, D], f32r, kind="ExternalInput").ap()
    qT_d = nc.dram_tensor("qT", [HEAD_DIM, QCOLS], bf16,
                          kind="ExternalInput").ap()
    mask_d = nc.dram_tensor("mask", [128, T * BS], bf16,
                            kind="ExternalInput").ap()
    ident_d = nc.dram_tensor("ident", [128, 128], bf16,
                             kind="ExternalInput").ap()
    # num packs PV and den: per head 129 cols = 128 d + 1 denominator
    num_d = nc.dram_tensor("num", [64, H * 129], f32,
                           kind="ExternalOutput").ap()

    # chunk schedule: (dram row offset, rows-per-partition)
    chunks = []
    off = 0
    for r in RS:
        chunks.append((off, r))
        off += 128 * r

    with tile.TileContext(nc) as tc:
        with tc.tile_pool(name="const", bufs=1) as constp, \
             tc.tile_pool(name="kbuf", bufs=2) as kpool, \
             tc.tile_pool(name="vbuf", bufs=2) as vpool, \
             tc.tile_pool(name="k16", bufs=3) as k16p, \
             tc.tile_pool(name="ktsb", bufs=2) as ktp, \
             tc.tile_pool(name="psb", bufs=4) as ppool, \
             tc.tile_pool(name="outs", bufs=1) as outp, \
             tc.tile_pool(name="ps_kt", bufs=2, space="PSUM") as ps_kt, \
             tc.tile_pool(name="ps_s", bufs=2, space="PSUM") as ps_s, \
             tc.tile_pool(name="ps_pv", bufs=1, space="PSUM") as ps_pv:

            qt_sb = constp.tile([128, QCOLS], bf16)
            nc.sync.dma_start(qt_sb[:], qT_d)
            mask_sb = constp.tile([128, T * BS], bf16)
            nc.sync.dma_start(mask_sb[:], mask_d)
            ident = constp.tile([128, 128], bf16)
            nc.sync.dma_start(ident[:], ident_d)

            # v16 double buffers are persistent so the appended ones-columns
            # (den producers) are initialized exactly once
            v16 = [constp.tile([128, H, 129], bf16, name=f"v16_{i}")
                   for i in range(2)]
            for i in range(2):
                nc.vector.memset(v16[i][:, :, 128], 1.0)

            # pv[bg, h, 0:128] = numerator, pv[bg, h, 128] = denominator.
            # 129*4B per head: 3 heads fit a 2KB PSUM bank
            pvs = [ps_pv.tile([64, 3, 129], f32, name="pvA"),
                   ps_pv.tile([64, 3, 129], f32, name="pvB"),
                   ps_pv.tile([64, 2, 129], f32, name="pvC")]
            for t_ in pvs:
                nc.vector.memset(t_[:], 0.0)

            def pv_out(h):
                return pvs[h // 3][:, h % 3, :]

            def conv_k(kbuf, j):
                k16 = k16p.tile([128, D], bf16, tag="k16")
                nc.scalar.copy(k16[:], kbuf[:, j, :])
                return k16

            def conv_v(vbuf, j, t):
                dst = v16[t % 2]
                nc.vector.tensor_copy(
                    dst[:, :, 0:128],
                    vbuf[:, j, :].rearrange("p (h d) -> p h d", d=128))
                return dst

            def group(k16, vv, t):
                """one 128-slot group (bf16 K tile k16, bf16 V+ones vv)."""
                ktps = ps_kt.tile([128, H, 128], bf16, tag="ktps")
                for h in range(H):
                    nc.tensor.transpose(
                        ktps[:, h, :], k16[:, h * 128:(h + 1) * 128],
                        ident[:])
                ktsb = ktp.tile([128, H, 128], bf16, tag="kt")
                nc.vector.tensor_copy(ktsb[:, 0:4, :], ktps[:, 0:4, :])
                nc.scalar.copy(ktsb[:, 4:8, :], ktps[:, 4:8, :])

                # scores[slot, (h,b,g)] = sum_d K[slot,d_h] Q[(b,g),d]
                scores = ps_s.tile([128, QCOLS], f32, tag="scores")
                for h in range(H):
                    nc.tensor.matmul(
                        out=scores[:, h * 64:(h + 1) * 64],
                        lhsT=ktsb[:, h, :],
                        rhs=qt_sb[:, h * 64:(h + 1) * 64],
                        start=True, stop=True)

                p_sb = ppool.tile([128, QCOLS], bf16, tag="p")
                nc.scalar.activation(p_sb[:], scores[:], EXP, scale=SCALE)

                # ownership weights: pm[slot, (h,b,g)] = p * mask[slot, b]
                pm = ppool.tile([128, QCOLS], bf16, tag="pm")
                mcols = mask_sb[:, t * BS:(t + 1) * BS]
                if mask_4d:
                    nc.vector.tensor_tensor(
                        out=pm[:].rearrange("p (h b g) -> p h b g",
                                            h=H, b=BS, g=GROUP),
                        in0=p_sb[:].rearrange("p (h b g) -> p h b g",
                                              h=H, b=BS, g=GROUP),
                        in1=mcols.unsqueeze(1).unsqueeze(3).broadcast_to(
                            [128, H, BS, GROUP]),
                        op=MULT)
                else:
                    for h in range(H):
                        nc.vector.tensor_tensor(
                            out=pm[:, h * 64:(h + 1) * 64].rearrange(
                                "p (b g) -> p b g", g=GROUP),
                            in0=p_sb[:, h * 64:(h + 1) * 64].rearrange(
                                "p (b g) -> p b g", g=GROUP),
                            in1=mcols.unsqueeze(2).broadcast_to(
                                [128, BS, GROUP]),
                            op=MULT)

                # num[bg, h, d] += sum_slot pm[slot, bg] [V | 1][slot, d]
                for h in range(H):
                    nc.tensor.matmul(
                        out=pv_out(h),
                        lhsT=pm[:, h * 64:(h + 1) * 64],
                        rhs=vv[:, h, :],
                        start=False, stop=False, skip_group_check=True)

            # software pipeline: convert group t+1 while computing group t
    # (k16/v16 conversions sit between each group's DMA and its matmuls)
            todo = []      # (k16, vv, t) ready for compute
            t = 0
            for off, r in chunks + [(None, 1)]:
                if off is not None:
                    kbuf = kpool.tile([128, RMAX, D], f32r, tag="k")
                    nc.sync.dma_start(
                        kbuf[:, 0:r, :].rearrange("p j d -> p (j d)"),
                        kc[off:off + 128 * r, :].rearrange(
                            "(p j) d -> p (j d)", j=r))
                    vbuf = vpool.tile([128, RMAX, D], f32r, tag="v")
                    nc.scalar.dma_start(
                        vbuf[:, 0:r, :].rearrange("p j d -> p (j d)"),
                        vc[off:off + 128 * r, :].rearrange(
                            "(p j) d -> p (j d)", j=r))
                else:
                    kbuf = kpool.tile([128, 1, D], f32r, tag="knew")
                    nc.sync.dma_start(kbuf[:, 0, :], knew_d)
                    vbuf = vpool.tile([128, 1, D], f32r, tag="vnew")
                    nc.scalar.dma_start(vbuf[:, 0, :], vnew_d)
                for j in range(r):
                    todo.append((conv_k(kbuf, j), conv_v(vbuf, j, t), t))
                    t += 1
                    if len(todo) >= 2:
                        group(*todo.pop(0))
            for args in todo:
                group(*args)

            onum = outp.tile([64, H, 129], f32)
            for h in range(H):
                eng = nc.vector if h % 2 == 0 else nc.scalar
                (eng.tensor_copy if h % 2 == 0 else eng.copy)(
                    onum[:, h, :], pv_out(h))
            nc.sync.dma_start(
                num_d, onum[:].rearrange("p h d -> p (h d)"))

    nc.compile()
    return nc


def shard_inputs(q, k, v, k_cache, v_cache, slot_mapping, page_indices):
    """Host-side sharding: contiguous zero-copy cache slices per core plus
    small index-derived tensors (masks, transposed queries, new-token tiles)."""
    q = np.ascontiguousarray(np.asarray(q, dtype=np.float32))
    k = np.ascontiguousarray(np.asarray(k, dtype=np.float32))
    v = np.ascontiguousarray(np.asarray(v, dtype=np.float32))
    k_cache = np.asarray(k_cache, dtype=np.float32)
    v_cache = np.asarray(v_cache, dtype=np.float32)
    slot_mapping = np.asarray(slot_mapping, dtype=np.int64).ravel()
    page_indices = np.asarray(page_indices, dtype=np.int64)

    # qT[d, (h, b, g)] = q[b, h*GROUP+g, d]
    qr = q.reshape(BS, NUM_KV_HEADS, GROUP, HEAD_DIM)
    qT = np.ascontiguousarray(
        qr.transpose(3, 1, 0, 2).reshape(HEAD_DIM, QCOLS)
    ).astype(ml_dtypes.bfloat16)

    # ownership weights: count[slot, b] = multiplicity of slot in seq b's pages
    count = np.zeros((NUM_SLOTS, BS), dtype=np.float32)
    np.add.at(count,
              (page_indices.ravel(),
               np.repeat(np.arange(BS), KV_LEN)),
              1.0)
    # new-token slots: reference scatters k/v rows there BEFORE the gather, so
    # the streamed (old) row must contribute nothing; the appended kvnew tile
    # re-injects each referencing (seq, count) exactly once fleet-wide.
    # With duplicate slot_mapping entries the last writer wins (jax .at[].set).
    final_writer = {}
    for j in range(BS):
        final_writer[int(slot_mapping[j])] = j
    newcnt = np.zeros((BS, BS), dtype=np.float32)   # [kvnew row j, seq b]
    for s, j in final_writer.items():
        newcnt[j, :] = count[s, :]
        count[s, :] = 0.0

    # appended tile data: rows 0..15 = new k/v, rest zeros
    knew = np.zeros((128, D), dtype=np.float32)
    knew[:BS] = k
    vnew = np.zeros((128, D), dtype=np.float32)
    vnew[:BS] = v

    # streamed slot (group t from chunk (off, r), partition p, sub j)
    #   = off + p*r + j ; group index t advances j-major within a chunk
    perm = np.empty(SLOTS_PER_CORE, dtype=np.int64)
    gi = 0
    off = 0
    for r in RS:
        idx = off + np.arange(128)[:, None] * r + np.arange(r)[None, :]
        for j in range(r):
            perm[gi * 128:(gi + 1) * 128] = idx[:, j]
            gi += 1
        off += 128 * r

    in_maps = []
    for c in range(N_CORES):
        base = c * SLOTS_PER_CORE
        mcore = count[base:base + SLOTS_PER_CORE][perm]     # [4096, 16]
        m = np.zeros((128, T * BS), dtype=np.float32)
        m[:, :NT * BS] = (
            mcore.reshape(NT, 128, BS).transpose(1, 0, 2)
            .reshape(128, NT * BS))
        # new-token tile mask: kvnew row j handled by core j % N_CORES
        for j in range(BS):
            if j % N_CORES == c:
                m[j, NT * BS:(NT + 1) * BS] = newcnt[j]
        in_maps.append({
            "k_shard": k_cache[base:base + SLOTS_PER_CORE],
            "v_shard": v_cache[base:base + SLOTS_PER_CORE],
            "k_new": knew,
            "v_new": vnew,
            "qT": qT,
            "mask": m.astype(ml_dtypes.bfloat16),
            "ident": np.eye(128, dtype=np.float32).astype(ml_dtypes.bfloat16),
        })
    return in_maps


_PROGS = {}
last_results = None  # BassKernelResults of the most recent kernel() call


def kernel(q, k, v, k_cache, v_cache, slot_mapping, page_indices):
    global last_results
    from concourse.bass_utils import run_bass_kernel_spmd

    in_maps = shard_inputs(q, k, v, k_cache, v_cache, slot_mapping,
                           page_indices)
    if "prog" not in _PROGS:
        try:
            _PROGS["prog"] = build_program(mask_4d=True)
        except Exception:
            _PROGS["prog"] = build_program(mask_4d=False)
    res = run_bass_kernel_spmd(_PROGS["prog"], in_maps,
                               core_ids=list(range(N_CORES)))
    last_results = res

    acc = np.zeros((64, H, 129), dtype=np.float64)
    for c in range(N_CORES):
        acc += res.results[c]["num"].astype(np.float64).reshape(64, H, 129)
    num = acc[:, :, 0:128]                      # [(b,g), h, d]
    den = acc[:, :, 128]                        # [(b,g), h]
    o = num / den[:, :, None]
    o = o.reshape(BS, GROUP, NUM_KV_HEADS, HEAD_DIM)   # [b, g, h, d]
    out = o.transpose(0, 2, 1, 3).reshape(BS, NUM_HEADS * HEAD_DIM)
    return np.ascontiguousarray(out.astype(np.float32))


# revision 6
# speedup vs baseline: 1.3962x; 1.1110x over previous
"""Paged GQA decode attention (sparse_attention) on 8 TRN2 NeuronCores.

Slot-parallel streaming design: the page table is (in the graded problem) a
permutation of all 32768 cache slots, so the full K/V caches are read exactly
once.  Core i owns the contiguous slot range [i*4096, (i+1)*4096) and STREAMS
it sequentially at full HBM bandwidth in chunks laid out
[128 partitions, r rows, 1024] (16KB contiguous DRAM per partition per
descriptor at r=4; chunks ramp small->large->small to fill and drain the
pipeline quickly).  K streams on the SP HWDGE queue, V on the gpsimd SWDGE
queue so neither compute engine pays DMA-issue time.

Each core scores its slots against ALL 16 sequences' queries, multiplies by a
host-built per-(slot, seq) ownership weight (0/1 under the permutation;
counts for general inputs), and accumulates the partial attention numerator
and denominator in PSUM over the whole stream.  The 16 new-token k/v rows
arrive as a host-prebuilt bf16 tile (zero-padded, ones-column included) whose
mask rows inject each new token exactly once across the fleet; the streamed
mask row of a replaced slot is zeroed on the core that owns it.  No gathers,
no scatters -- one fully static program per core.  The host sums the per-core
partials (flash-decoding split-K combine) and normalizes.

Per-instruction HW profiling showed the tensor engine is cadence-bound at
roughly max(ldweights, matmul-stream) ~ 1ns/column, so the pipeline minimizes
PE columns moved:
  - everything on the PE is bf16: K tiles are converted f32->bf16 on the
    Vector engine and V tiles on the Activation engine, two groups ahead of
    their consumers (f32r narrow matmuls run at 4 cycles/row and f32r weight
    loads at ~2 cycles/column),
  - PV is flipped (lhsT = masked-P, 64-column weight loads; V is the moving
    operand) and V carries an appended ones-column per head so each PV
    matmul also emits that head's denominator -- no separate den matmul,
  - PSUM accumulators are memset once and PV runs start=stop=False
    (skip_group_check): TRN2 zeroes lazily per 2KB region on
    start_tensor_calc, so 8 per-head groups in one bank would clobber each
    other.
"""

import ml_dtypes
import numpy as np

# ---- problem constants (must match the harness's reference.py) ----
NUM_HEADS = 32
NUM_KV_HEADS = 8
HEAD_DIM = 128
BS = 16
KV_LEN = 2048
NUM_SLOTS = BS * KV_LEN          # 32768
D = NUM_KV_HEADS * HEAD_DIM      # 1024 (cache row width, f32)
SCALE = HEAD_DIM ** -0.5
N_CORES = 8
GROUP = NUM_HEADS // NUM_KV_HEADS  # 4

SLOTS_PER_CORE = NUM_SLOTS // N_CORES   # 4096
RMAX = 4                                # max DRAM rows per partition per chunk
# per-chunk rows/partition: small chunks at both ends for pipeline fill/drain
RS = [1, 1, 2, 2, 4, 4, 4, 4, 4, 2, 2, 1, 1]
assert sum(RS) == SLOTS_PER_CORE // 128
NT = SLOTS_PER_CORE // 128              # 32 streamed 128-slot groups
T = NT + 1                              # + appended new-token group
QCOLS = NUM_KV_HEADS * BS * GROUP       # 512 score columns: (h, b, g)
H = NUM_KV_HEADS
PIPE = 2                                # conversions run PIPE groups ahead


def build_program(mask_4d=True):
    import concourse.bacc as bacc
    import concourse.mybir as mybir
    import concourse.tile as tile

    f32 = mybir.dt.float32
    f32r = mybir.dt.float32r
    bf16 = mybir.dt.bfloat16
    EXP = mybir.ActivationFunctionType.Exp
    MULT = mybir.AluOpType.mult

    nc = bacc.Bacc("TRN2", target_bir_lowering=False, debug=False,
                   enable_asserts=False, num_devices=N_CORES,
                   num_swdge_queues=1)

    kc = nc.dram_tensor("k_shard", [SLOTS_PER_CORE, D], f32r,
                        kind="ExternalInput").ap()
    vc = nc.dram_tensor("v_shard", [SLOTS_PER_CORE, D], f32r,
                        kind="ExternalInput").ap()
    knew_d = nc.dram_tensor("k_new16", [128, D], bf16,
                            kind="ExternalInput").ap()
    vnew_d = nc.dram_tensor("v_new16", [128, H * 129], bf16,
                            kind="ExternalInput").ap()
    qT_d = nc.dram_tensor("qT", [HEAD_DIM, QCOLS], bf16,
                          kind="ExternalInput").ap()
    mask_d = nc.dram_tensor("mask", [128, T * BS], bf16,
                            kind="ExternalInput").ap()
    ident_d = nc.dram_tensor("ident", [128, 128], bf16,
                             kind="ExternalInput").ap()
    # num packs PV and den: per head 129 cols = 128 d + 1 denominator
    num_d = nc.dram_tensor("num", [64, H * 129], f32,
                           kind="ExternalOutput").ap()

    chunks = []
    off = 0
    for r in RS:
        chunks.append((off, r))
        off += 128 * r

    with tile.TileContext(nc) as tc:
        with tc.tile_pool(name="const", bufs=1) as constp, \
             tc.tile_pool(name="kbuf", bufs=3) as kpool, \
             tc.tile_pool(name="vbuf", bufs=3) as vpool, \
             tc.tile_pool(name="k16", bufs=3) as k16p, \
             tc.tile_pool(name="ktsb", bufs=2) as ktp, \
             tc.tile_pool(name="psb", bufs=4) as ppool, \
             tc.tile_pool(name="outs", bufs=1) as outp, \
             tc.tile_pool(name="ps_kt", bufs=2, space="PSUM") as ps_kt, \
             tc.tile_pool(name="ps_s", bufs=3, space="PSUM") as ps_s, \
             tc.tile_pool(name="ps_pv", bufs=1, space="PSUM") as ps_pv:

            qt_sb = constp.tile([128, QCOLS], bf16)
            nc.sync.dma_start(qt_sb[:], qT_d)
            mask_sb = constp.tile([128, T * BS], bf16)
            nc.sync.dma_start(mask_sb[:], mask_d)
            ident = constp.tile([128, 128], bf16)
            nc.sync.dma_start(ident[:], ident_d)
            knew16 = constp.tile([128, D], bf16)
            nc.sync.dma_start(knew16[:], knew_d)
            vnew16 = constp.tile([128, H, 129], bf16)
            nc.sync.dma_start(vnew16[:].rearrange("p h d -> p (h d)"), vnew_d)

            # v16 rotation is persistent so the appended ones-columns
            # (denominator producers) are initialized exactly once
            v16 = [constp.tile([128, H, 129], bf16, name=f"v16_{i}")
                   for i in range(3)]
            for i in range(3):
                nc.vector.memset(v16[i][:, :, 128], 1.0)

            # pv[bg, h, 0:128] = numerator, pv[bg, h, 128] = denominator.
            # 129*4B per head: 3 heads fit a 2KB PSUM bank
            pvs = [ps_pv.tile([64, 3, 129], f32, name="pvA"),
                   ps_pv.tile([64, 3, 129], f32, name="pvB"),
                   ps_pv.tile([64, 2, 129], f32, name="pvC")]
            for t_ in pvs:
                nc.vector.memset(t_[:], 0.0)

            def pv_out(h):
                return pvs[h // 3][:, h % 3, :]

            def conv(kbuf, vbuf, j, t):
                k16 = k16p.tile([128, D], bf16, tag="k16")
                nc.vector.tensor_copy(k16[:], kbuf[:, j, :])
                vv = v16[t % 3]
                nc.scalar.copy(
                    vv[:, :, 0:128],
                    vbuf[:, j, :].rearrange("p (h d) -> p h d", d=128))
                return k16, vv

            def group(k16, vv, t):
                """one 128-slot group (bf16 K tile k16, bf16 V+ones vv)."""
                ktps = ps_kt.tile([128, H, 128], bf16, tag="ktps")
                for h in range(H):
                    nc.tensor.transpose(
                        ktps[:, h, :], k16[:, h * 128:(h + 1) * 128],
                        ident[:])
                ktsb = ktp.tile([128, H, 128], bf16, tag="kt")
                nc.vector.tensor_copy(ktsb[:, 0:4, :], ktps[:, 0:4, :])
                nc.scalar.copy(ktsb[:, 4:8, :], ktps[:, 4:8, :])

                # scores[slot, (h,b,g)] = sum_d K[slot,d_h] Q[(b,g),d]
                scores = ps_s.tile([128, QCOLS], f32, tag="scores")
                for h in range(H):
                    nc.tensor.matmul(
                        out=scores[:, h * 64:(h + 1) * 64],
                        lhsT=ktsb[:, h, :],
                        rhs=qt_sb[:, h * 64:(h + 1) * 64],
                        start=True, stop=True)

                p_sb = ppool.tile([128, QCOLS], bf16, tag="p")
                nc.scalar.activation(p_sb[:], scores[:], EXP, scale=SCALE)

                # ownership weights: pm[slot, (h,b,g)] = p * mask[slot, b]
                pm = ppool.tile([128, QCOLS], bf16, tag="pm")
                mcols = mask_sb[:, t * BS:(t + 1) * BS]
                if mask_4d:
                    nc.vector.tensor_tensor(
                        out=pm[:].rearrange("p (h b g) -> p h b g",
                                            h=H, b=BS, g=GROUP),
                        in0=p_sb[:].rearrange("p (h b g) -> p h b g",
                                              h=H, b=BS, g=GROUP),
                        in1=mcols.unsqueeze(1).unsqueeze(3).broadcast_to(
                            [128, H, BS, GROUP]),
                        op=MULT)
                else:
                    for h in range(H):
                        nc.vector.tensor_tensor(
                            out=pm[:, h * 64:(h + 1) * 64].rearrange(
                                "p (b g) -> p b g", g=GROUP),
                            in0=p_sb[:, h * 64:(h + 1) * 64].rearrange(
                                "p (b g) -> p b g", g=GROUP),
                            in1=mcols.unsqueeze(2).broadcast_to(
                                [128, BS, GROUP]),
                            op=MULT)

                # num[bg, h, d] += sum_slot pm[slot, bg] [V | 1][slot, d]
                for h in range(H):
                    nc.tensor.matmul(
                        out=pv_out(h),
                        lhsT=pm[:, h * 64:(h + 1) * 64],
                        rhs=vv[:, h, :],
                        start=False, stop=False, skip_group_check=True)

            # software pipeline: conversions run PIPE groups ahead of compute
            chunk_iter = iter(chunks)
            avail = []          # (kbuf, vbuf, j) with pending conversions
            ready = {}          # t -> (k16, vv)
            n_conv = 0

            def pump():
                off, r = next(chunk_iter)
                kbuf = kpool.tile([128, RMAX, D], f32r, tag="k")
                nc.sync.dma_start(
                    kbuf[:, 0:r, :].rearrange("p j d -> p (j d)"),
                    kc[off:off + 128 * r, :].rearrange(
                        "(p j) d -> p (j d)", j=r))
                vbuf = vpool.tile([128, RMAX, D], f32r, tag="v")
                nc.gpsimd.dma_start(
                    vbuf[:, 0:r, :].rearrange("p j d -> p (j d)"),
                    vc[off:off + 128 * r, :].rearrange(
                        "(p j) d -> p (j d)", j=r))
                avail.extend((kbuf, vbuf, j) for j in range(r))

            for t in range(T):
                while n_conv <= min(t + PIPE, T - 1):
                    if n_conv == NT:
                        ready[NT] = (knew16, vnew16)
                    else:
                        if not avail:
                            pump()
                        ready[n_conv] = conv(*avail.pop(0), n_conv)
                    n_conv += 1
                group(*ready.pop(t), t)

            onum = outp.tile([64, H, 129], f32)
            for h in range(H):
                if h % 2 == 0:
                    nc.vector.tensor_copy(onum[:, h, :], pv_out(h))
                else:
                    nc.scalar.copy(onum[:, h, :], pv_out(h))
            nc.sync.dma_start(
                num_d, onum[:].rearrange("p h d -> p (h d)"))

    nc.compile()
    return nc


def shard_inputs(q, k, v, k_cache, v_cache, slot_mapping, page_indices):
    """Host-side sharding: contiguous zero-copy cache slices per core plus
    small index-derived tensors (masks, transposed queries, new-token tiles)."""
    q = np.ascontiguousarray(np.asarray(q, dtype=np.float32))
    k = np.ascontiguousarray(np.asarray(k, dtype=np.float32))
    v = np.ascontiguousarray(np.asarray(v, dtype=np.float32))
    k_cache = np.asarray(k_cache, dtype=np.float32)
    v_cache = np.asarray(v_cache, dtype=np.float32)
    slot_mapping = np.asarray(slot_mapping, dtype=np.int64).ravel()
    page_indices = np.asarray(page_indices, dtype=np.int64)

    # qT[d, (h, b, g)] = q[b, h*GROUP+g, d]
    qr = q.reshape(BS, NUM_KV_HEADS, GROUP, HEAD_DIM)
    qT = np.ascontiguousarray(
        qr.transpose(3, 1, 0, 2).reshape(HEAD_DIM, QCOLS)
    ).astype(ml_dtypes.bfloat16)

    # ownership weights: count[slot, b] = multiplicity of slot in seq b's pages
    count = np.zeros((NUM_SLOTS, BS), dtype=np.float32)
    np.add.at(count,
              (page_indices.ravel(),
               np.repeat(np.arange(BS), KV_LEN)),
              1.0)
    # new-token slots: reference scatters k/v rows there BEFORE the gather, so
    # the streamed (old) row must contribute nothing; the appended kvnew tile
    # re-injects each referencing (seq, count) exactly once fleet-wide.
    # With duplicate slot_mapping entries the last writer wins (jax .at[].set).
    final_writer = {}
    for j in range(BS):
        final_writer[int(slot_mapping[j])] = j
    newcnt = np.zeros((BS, BS), dtype=np.float32)   # [kvnew row j, seq b]
    for s, j in final_writer.items():
        newcnt[j, :] = count[s, :]
        count[s, :] = 0.0

    # appended tile data, host-converted to the device layouts
    knew = np.zeros((128, D), dtype=np.float32)
    knew[:BS] = k
    knew16 = knew.astype(ml_dtypes.bfloat16)
    vnew16 = np.zeros((128, H, 129), dtype=np.float32)
    vnew16[:BS, :, 0:128] = v.reshape(BS, H, HEAD_DIM)
    vnew16[:, :, 128] = 1.0
    vnew16 = np.ascontiguousarray(
        vnew16.reshape(128, H * 129)).astype(ml_dtypes.bfloat16)

    # streamed slot (group t from chunk (off, r), partition p, sub j)
    #   = off + p*r + j ; group index t advances j-major within a chunk
    perm = np.empty(SLOTS_PER_CORE, dtype=np.int64)
    gi = 0
    off = 0
    for r in RS:
        idx = off + np.arange(128)[:, None] * r + np.arange(r)[None, :]
        for j in range(r):
            perm[gi * 128:(gi + 1) * 128] = idx[:, j]
            gi += 1
        off += 128 * r

    in_maps = []
    for c in range(N_CORES):
        base = c * SLOTS_PER_CORE
        mcore = count[base:base + SLOTS_PER_CORE][perm]     # [4096, 16]
        m = np.zeros((128, T * BS), dtype=np.float32)
        m[:, :NT * BS] = (
            mcore.reshape(NT, 128, BS).transpose(1, 0, 2)
            .reshape(128, NT * BS))
        # new-token tile mask: kvnew row j handled by core j % N_CORES
        for j in range(BS):
            if j % N_CORES == c:
                m[j, NT * BS:(NT + 1) * BS] = newcnt[j]
        in_maps.append({
            "k_shard": k_cache[base:base + SLOTS_PER_CORE],
            "v_shard": v_cache[base:base + SLOTS_PER_CORE],
            "k_new16": knew16,
            "v_new16": vnew16,
            "qT": qT,
            "mask": m.astype(ml_dtypes.bfloat16),
            "ident": np.eye(128, dtype=np.float32).astype(ml_dtypes.bfloat16),
        })
    return in_maps


_PROGS = {}
last_results = None  # BassKernelResults of the most recent kernel() call


def kernel(q, k, v, k_cache, v_cache, slot_mapping, page_indices):
    global last_results
    from concourse.bass_utils import run_bass_kernel_spmd

    in_maps = shard_inputs(q, k, v, k_cache, v_cache, slot_mapping,
                           page_indices)
    if "prog" not in _PROGS:
        try:
            _PROGS["prog"] = build_program(mask_4d=True)
        except Exception:
            _PROGS["prog"] = build_program(mask_4d=False)
    res = run_bass_kernel_spmd(_PROGS["prog"], in_maps,
                               core_ids=list(range(N_CORES)))
    last_results = res

    acc = np.zeros((64, H, 129), dtype=np.float64)
    for c in range(N_CORES):
        acc += res.results[c]["num"].astype(np.float64).reshape(64, H, 129)
    num = acc[:, :, 0:128]                      # [(b,g), h, d]
    den = acc[:, :, 128]                        # [(b,g), h]
    o = num / den[:, :, None]
    o = o.reshape(BS, GROUP, NUM_KV_HEADS, HEAD_DIM)   # [b, g, h, d]
    out = o.transpose(0, 2, 1, 3).reshape(BS, NUM_HEADS * HEAD_DIM)
    return np.ascontiguousarray(out.astype(np.float32))
